# revision 14
# baseline (speedup 1.0000x reference)
"""DeepseekECMoE (expert-choice MoE) Trainium2 kernel, 8-way expert-parallel.

Layout per core c (SPMD, differences only via inputs):
  - gate for OWN batch only (f32r matmul) -> full softmax [E, S] ->
    AllToAll exchanges affinity rows so core c holds [batch, S] for
    expert c -> exact top-256 per batch via max8/max_index/match_replace
    -> token gather via indirect DMA -> expert MLP (bf16 matmuls,
    erf-gelu on ACT), expert weights loaded once -> score-weighted bf16
    token outputs + indices out.
  - shared expert for batch b=c (bf16 matmuls), bf16 output.
Emission order overlaps the serial top-k DVE chain with the
shared-expert down projection on PE, and the AllToAll with the
shared-expert gate/up phase.
Host combines: scatter-add weighted expert outputs, transpose, add shared.
"""
import numpy as np
import ml_dtypes

import concourse.bass as bass
import concourse.tile as tile
from concourse import bacc, bass_isa, mybir
from concourse.bass2jax import install_neuronx_cc_hook, _bass_exec_p, partition_id_tensor
from concourse.masks import make_identity

B, S, H, E = 8, 1024, 1024, 8
I, ISH, CAP = 2048, 2048, 256
P = 128
HC, SC, NI, NISH = H // P, S // P, I // P, ISH // P
N_CORES = 8
dt = mybir.dt
BF16 = ml_dtypes.bfloat16

_CACHE: dict = {}


def _build_nc(act_name="Gelu"):
    nc = bacc.Bacc("TRN2", target_bir_lowering=False, debug=False,
                   num_devices=N_CORES)

    # ---- DRAM I/O ----
    hidTc = nc.dram_tensor("hidTc", [H, S], dt.float32r, kind="ExternalInput")
    hidf = nc.dram_tensor("hidf", [B * S, H], dt.bfloat16, kind="ExternalInput")
    gw = nc.dram_tensor("gw", [P, HC * E], dt.float32r, kind="ExternalInput")
    # gate/up weights packed in i-pairs: [2(g|u), NI//2, P, 2*HC*P]
    gut = nc.dram_tensor("gut", [2, NI // 2, P, 2 * HC * P], dt.bfloat16,
                         kind="ExternalInput")
    sgut = nc.dram_tensor("sgut", [2, NISH // 2, P, 2 * HC * P], dt.bfloat16,
                          kind="ExternalInput")
    # down weights packed partition-major: [P, NI*H]
    dpP = nc.dram_tensor("dpP", [P, NI * H], dt.bfloat16, kind="ExternalInput")
    sdP = nc.dram_tensor("sdP", [P, NISH * H], dt.bfloat16, kind="ExternalInput")

    w_out = nc.dram_tensor("w_out", [B, CAP, H], dt.bfloat16, kind="ExternalOutput")
    idxo = nc.dram_tensor("idxo", [B, CAP], dt.uint32, kind="ExternalOutput")
    sh_out = nc.dram_tensor("sh_out", [S, H], dt.bfloat16, kind="ExternalOutput")

    aff_send = nc.dram_tensor("aff_send", [E, S], dt.float32)
    aff_recv = nc.dram_tensor("aff_recv", [E, S], dt.float32)

    AF = mybir.ActivationFunctionType
    ACT = getattr(AF, act_name)
    from contextlib import ExitStack
    with tile.TileContext(nc) as tc:
        with ExitStack() as ctx:
            pool = lambda name, bufs, **kw: ctx.enter_context(
                tc.tile_pool(name=name, bufs=bufs, **kw))
            pconst = pool("consts", 1)
            phts = pool("hts", 4)
            pgate = pool("gate", 1)
            proute = pool("route", 1)
            phsh = pool("hsh", 8)
            pguw = pool("guw", 6)
            pacts = pool("acts", 16)
            pdw = pool("dw", 1)
            ptokT = pool("tokT", 8)
            pgath = pool("gath", 3)
            pgel = pool("gel", 2)
            pwo = pool("wo", 4)
            psmallT = pool("smallT", 2)
            # PSUM: 8 banks total
            pps = pool("pps", 2, space="PSUM")     # gate + down proj
            pgu = pool("pgu", 2, space="PSUM")     # gate/up matmuls (pg+pu tags)
            ptr = pool("ptr", 2, space="PSUM")     # transposes

            # ---- constants ----
            t_gw = pconst.tile([P, HC * E], dt.float32r)
            nc.sync.dma_start(t_gw[:], gw[:])
            t_idb = pconst.tile([P, P], dt.bfloat16)
            make_identity(nc, t_idb[:])
            t_idf8 = pconst.tile([E, E], dt.float32)
            make_identity(nc, t_idf8[:])
            t_iotB = pconst.tile([E, 1], dt.int32)
            nc.gpsimd.iota(t_iotB[:], pattern=[[0, 1]], base=0,
                           channel_multiplier=S)
            t_iotBf = pconst.tile([E, 1], dt.float32)
            nc.vector.tensor_copy(t_iotBf[:], t_iotB[:])

            # ---- early weight loads (scalar HWDGE; sync carries the rest) ----
            sdw = pdw.tile([P, NISH * H], dt.bfloat16, tag="dw", name="sdw")
            for k in range(8):
                nc.sync.dma_start(sdw[:, k * 2 * H:(k + 1) * 2 * H],
                                    sdP[:, k * 2 * H:(k + 1) * 2 * H])

            # ---- gate for own batch -> softmax all experts ----
            hsh = []
            pl = [pps.tile([P, 512], dt.float32, tag="pps", name="pl0"),
                  pps.tile([P, 512], dt.float32, tag="pps", name="pl1")]
            for hc in range(HC):
                ht = phts.tile([P, S], dt.float32r)
                nc.sync.dma_start(ht[:], hidTc[hc * P:(hc + 1) * P, :])
                t = phsh.tile([P, S], dt.bfloat16, tag="hsh", name="hsh")
                nc.vector.tensor_copy(t[:], ht[:])
                hsh.append(t)
                for sblk in range(2):
                    nc.tensor.matmul(pl[sblk][:E], t_gw[:, hc * E:(hc + 1) * E],
                                     ht[:, sblk * 512:(sblk + 1) * 512],
                                     start=(hc == 0), stop=(hc == HC - 1))
            exp_b = pgate.tile([E, S], dt.float32, tag="exp", name="exp_b")
            for sblk in range(2):
                nc.scalar.activation(exp_b[:, sblk * 512:(sblk + 1) * 512],
                                     pl[sblk][:E], AF.Exp)
            den8 = pgate.tile([E, S], dt.float32, tag="den", name="den8")
            nc.gpsimd.partition_all_reduce(den8[:], exp_b[:], channels=E,
                                           reduce_op=bass_isa.ReduceOp.add)
            nc.vector.reciprocal(den8[:], den8[:])
            aff = pgate.tile([E, S], dt.float32, tag="aff", name="aff")
            nc.vector.tensor_mul(aff[:], exp_b[:], den8[:])
            nc.sync.dma_start(aff_send[:], aff[:])
            nc.gpsimd.collective_compute(
                "AllToAll", mybir.AluOpType.bypass,
                replica_groups=[list(range(N_CORES))],
                ins=[aff_send[:].opt()], outs=[aff_recv[:].opt()])
            afftile = proute.tile([E, S], dt.float32)
            nc.sync.dma_start(afftile[:], aff_recv[:])

            # ---- shared expert gate/up ----
            actsh = []
            for j in range(NISH // 2):
                sgu = pguw.tile([P, 2 * HC * P], dt.bfloat16, name="sgu")
                nc.sync.dma_start(sgu[:], sgut[0, j])
                suu = pguw.tile([P, 2 * HC * P], dt.bfloat16, name="suu")
                nc.sync.dma_start(suu[:], sgut[1, j])
                for k in range(2):
                    sg = sgu[:, k * HC * P:(k + 1) * HC * P]
                    su = suu[:, k * HC * P:(k + 1) * HC * P]
                    a = pacts.tile([P, S], dt.bfloat16, tag="acts", name="acts")
                    for sblk in range(2):
                        pg = pgu.tile([P, 512], dt.float32, tag="pg", name="pg",
                                      bufs=2)
                        for hc in range(HC):
                            nc.tensor.matmul(
                                pg[:], sg[:, hc * P:(hc + 1) * P],
                                hsh[hc][:, sblk * 512:(sblk + 1) * 512],
                                start=(hc == 0), stop=(hc == HC - 1))
                        pu = pgu.tile([P, 512], dt.float32, tag="pu", name="pu",
                                      bufs=2)
                        for hc in range(HC):
                            nc.tensor.matmul(
                                pu[:], su[:, hc * P:(hc + 1) * P],
                                hsh[hc][:, sblk * 512:(sblk + 1) * 512],
                                start=(hc == 0), stop=(hc == HC - 1))
                        gel = pgel.tile([P, 512], dt.float32)
                        nc.scalar.activation(gel[:], pg[:], ACT)
                        nc.vector.tensor_mul(a[:, sblk * 512:(sblk + 1) * 512],
                                             gel[:], pu[:])
                    actsh.append(a)

            # ---- shared expert down (PE) ----
            for sblk in range(SC):
                for hh in range(2):
                    pd = pps.tile([P, 512], dt.float32, tag="pps", name="pd")
                    for ic in range(NISH):
                        nc.tensor.matmul(
                            pd[:], actsh[ic][:, sblk * P:(sblk + 1) * P],
                            sdw[:, ic * H + hh * 512:ic * H + hh * 512 + 512],
                            start=(ic == 0), stop=(ic == NISH - 1))
                    sho = pwo.tile([P, 512], dt.bfloat16, tag="wo", name="wo")
                    nc.scalar.copy(sho[:], pd[:])
                    nc.sync.dma_start(
                        sh_out[sblk * P:(sblk + 1) * P, hh * 512:(hh + 1) * 512],
                        sho[:])

            # ---- top-k (serial DVE chain; overlaps shared down on PE) ----
            t_scores = proute.tile([E, CAP], dt.float32)
            t_idxu = proute.tile([E, CAP], dt.uint32)
            t_idxf = proute.tile([E, CAP], dt.float32)
            t_idxg = proute.tile([E, CAP], dt.float32)
            for i in range(CAP // 8):
                sc8 = t_scores[:, i * 8:(i + 1) * 8]
                nc.vector.max(sc8, afftile[:])
                nc.vector.max_index(t_idxu[:, i * 8:(i + 1) * 8], sc8, afftile[:])
                nc.vector.match_replace(afftile[:], sc8, afftile[:], -1e30)
            nc.sync.dma_start(idxo[:], t_idxu[:])
            nc.vector.tensor_copy(t_idxf[:], t_idxu[:])
            nc.vector.tensor_scalar(t_idxg[:], t_idxf[:], t_iotBf[:, :1],
                                    None, mybir.AluOpType.add)

            # transpose scores + global indices to per-token columns
            scT = [psmallT.tile([P, E], dt.float32, tag="scT", name="scT")
                   for _ in range(2)]
            idxT = [psmallT.tile([P, E], dt.uint32, tag="idxT", name="idxT")
                    for _ in range(2)]
            for half in range(2):
                ptp = ptr.tile([P, P], dt.float32, tag="ptr", name="ptr")
                nc.tensor.transpose(ptp[:, :E],
                                    t_scores[:, half * P:(half + 1) * P],
                                    t_idf8[:])
                nc.vector.tensor_copy(scT[half][:], ptp[:, :E])
                ptq = ptr.tile([P, P], dt.float32, tag="ptr", name="ptr")
                nc.tensor.transpose(ptq[:, :E],
                                    t_idxg[:, half * P:(half + 1) * P],
                                    t_idf8[:])
                nc.vector.tensor_copy(idxT[half][:], ptq[:, :E])

            # ---- routed: two half-passes of 1024 tokens each ----
            dpw = pdw.tile([P, NI * H], dt.bfloat16, tag="dw", name="dpw")
            for k in range(8):
                nc.sync.dma_start(dpw[:, k * 2 * H:(k + 1) * 2 * H],
                                    dpP[:, k * 2 * H:(k + 1) * 2 * H])
            for half in range(2):
                # dispatch: indirect gather + PE transpose into tokT
                tokT = [ptokT.tile([P, 8 * P], dt.bfloat16, tag="tokT",
                                   name="tokT") for _ in range(HC)]
                for b in range(B):
                    g = pgath.tile([P, H], dt.bfloat16, tag="g", name="g")
                    nc.gpsimd.indirect_dma_start(
                        out=g[:], out_offset=None, in_=hidf[:],
                        in_offset=bass.IndirectOffsetOnAxis(
                            ap=idxT[half][:, b:b + 1], axis=0))
                    for hc in range(HC):
                        ptp = ptr.tile([P, P], dt.bfloat16, tag="ptr", name="ptr")
                        nc.tensor.transpose(ptp[:], g[:, hc * P:(hc + 1) * P],
                                            t_idb[:])
                        nc.scalar.copy(tokT[hc][:, b * P:(b + 1) * P], ptp[:])

                # gate/up (weights streamed; second half streams again)
                actT = []
                for j in range(NI // 2):
                    sgu = pguw.tile([P, 2 * HC * P], dt.bfloat16, name="sgu")
                    nc.sync.dma_start(sgu[:], gut[0, j])
                    suu = pguw.tile([P, 2 * HC * P], dt.bfloat16, name="suu")
                    nc.sync.dma_start(suu[:], gut[1, j])
                    for k in range(2):
                        sg = sgu[:, k * HC * P:(k + 1) * HC * P]
                        su = suu[:, k * HC * P:(k + 1) * HC * P]
                        a = pacts.tile([P, S], dt.bfloat16, tag="acts",
                                       name="acts")
                        for ch in range(2):
                            col = ch * 512
                            pg = pgu.tile([P, 512], dt.float32, tag="pg",
                                          name="pg", bufs=2)
                            for hc in range(HC):
                                nc.tensor.matmul(
                                    pg[:], sg[:, hc * P:(hc + 1) * P],
                                    tokT[hc][:, col:col + 512],
                                    start=(hc == 0), stop=(hc == HC - 1))
                            pu = pgu.tile([P, 512], dt.float32, tag="pu",
                                          name="pu", bufs=2)
                            for hc in range(HC):
                                nc.tensor.matmul(
                                    pu[:], su[:, hc * P:(hc + 1) * P],
                                    tokT[hc][:, col:col + 512],
                                    start=(hc == 0), stop=(hc == HC - 1))
                            gel = pgel.tile([P, 512], dt.float32)
                            nc.scalar.activation(gel[:], pg[:], ACT)
                            nc.vector.tensor_mul(a[:, col:col + 512],
                                                 gel[:], pu[:])
                        actT.append(a)

                # down + score weighting
                for b in range(B):
                    for hh in range(2):
                        pd = pps.tile([P, 512], dt.float32, tag="pps", name="pd")
                        for ic in range(NI):
                            nc.tensor.matmul(
                                pd[:], actT[ic][:, b * P:(b + 1) * P],
                                dpw[:, ic * H + hh * 512:ic * H + hh * 512 + 512],
                                start=(ic == 0), stop=(ic == NI - 1))
                        wo = pwo.tile([P, 512], dt.bfloat16, tag="wo", name="wo")
                        nc.vector.tensor_scalar(wo[:], pd[:],
                                                scT[half][:, b:b + 1],
                                                None, mybir.AluOpType.mult)
                        nc.sync.dma_start(
                            w_out[b, half * P:(half + 1) * P,
                                  hh * 512:(hh + 1) * 512], wo[:])

    nc.compile()
    return nc


class _Exec:
    """Cached multi-core PJRT executor (mirrors bass2jax.run_bass_via_pjrt)."""

    def __init__(self, nc):
        import jax
        from jax.sharding import Mesh, PartitionSpec
        from jax.experimental.shard_map import shard_map

        install_neuronx_cc_hook()
        self.nc = nc
        in_names, out_names, out_avals = [], [], []
        partition_name = (nc.partition_id_tensor.name
                          if nc.partition_id_tensor else None)
        for alloc in nc.m.functions[0].allocations:
            if not isinstance(alloc, mybir.MemoryLocationSet):
                continue
            if alloc.kind not in ("ExternalInput", "ExternalOutput"):
                continue
            name = alloc.memorylocations[0].name
            if alloc.kind == "ExternalInput":
                if name != partition_name:
                    in_names.append(name)
            elif alloc.kind == "ExternalOutput":
                out_names.append(name)
                out_avals.append(jax.core.ShapedArray(
                    tuple(alloc.tensor_shape), mybir.dt.np(alloc.dtype)))
        self.in_names, self.out_names, self.out_avals = in_names, out_names, out_avals
        self.partition_name = partition_name
        n_params = len(in_names)
        n_outs = len(out_names)
        all_in_names = list(in_names) + list(out_names)
        if partition_name is not None:
            all_in_names.append(partition_name)

        def _body(*args):
            operands = list(args)
            if partition_name is not None:
                operands.append(partition_id_tensor())
            outs = _bass_exec_p.bind(
                *operands,
                out_avals=tuple(out_avals),
                in_names=tuple(all_in_names),
                out_names=tuple(out_names),
                lowering_input_output_aliases=(),
                sim_require_finite=True,
                sim_require_nnan=True,
                nc=nc,
            )
            return tuple(outs)

        devices = jax.devices()[:N_CORES]
        mesh = Mesh(np.asarray(devices), ("core",))
        in_specs = (PartitionSpec("core"),) * (n_params + n_outs)
        out_specs = (PartitionSpec("core"),) * n_outs
        self.sharded = jax.jit(
            shard_map(_body, mesh=mesh, in_specs=in_specs, out_specs=out_specs,
                      check_rep=False),
            donate_argnums=tuple(range(n_params, n_params + n_outs)),
            keep_unused=True,
        )

    def concat_inputs(self, in_maps):
        return [
            np.concatenate([np.asarray(in_maps[c][name]) for c in range(N_CORES)],
                           axis=0)
            for name in self.in_names
        ]

    def zero_outs(self):
        return [np.zeros((N_CORES * a.shape[0], *a.shape[1:]), a.dtype)
                for a in self.out_avals]

    def run_raw(self, concat_in):
        return self.sharded(*concat_in, *self.zero_outs())

    def run(self, in_maps):
        out_arrs = self.run_raw(self.concat_inputs(in_maps))
        return [
            {name: np.asarray(out_arrs[i]).reshape(N_CORES, *self.out_avals[i].shape)[c]
             for i, name in enumerate(self.out_names)}
            for c in range(N_CORES)
        ]


def _get_exec():
    if "exec" not in _CACHE:
        _CACHE["exec"] = _Exec(_build_nc())
    return _CACHE["exec"]


def _prep_in_maps(hidden_states, gate_w, gate_proj, up_proj, down_proj,
                  s_gate, s_up, s_down):
    f32 = np.float32
    hid = np.ascontiguousarray(hidden_states, dtype=f32)
    hidT = np.ascontiguousarray(hid.transpose(0, 2, 1))
    hidf = np.ascontiguousarray(hid.reshape(B * S, H)).astype(BF16)
    gw = np.ascontiguousarray(
        np.asarray(gate_w, f32).reshape(HC, P, E).transpose(1, 0, 2).reshape(P, HC * E))

    def tile_gu(gT):  # gT [H, X] -> [X//(2P), P, 2*HC*P]  (i-pairs packed)
        X = gT.shape[1]
        t = gT.reshape(HC, P, X // P, P).transpose(2, 1, 0, 3)  # [X/P, P, HC, P]
        t = t.reshape(X // P, P, HC * P)
        return np.ascontiguousarray(
            t.reshape(X // (2 * P), 2, P, HC * P).transpose(0, 2, 1, 3)
            .reshape(X // (2 * P), P, 2 * HC * P))

    def pack_down(dT):  # dT [X, H] -> [P, (X//P)*H] partition-major
        X = dT.shape[0]
        return np.ascontiguousarray(
            dT.reshape(X // P, P, H).transpose(1, 0, 2).reshape(P, (X // P) * H))

    sgT = np.asarray(s_gate, f32).T  # [H, ISH]
    suT = np.asarray(s_up, f32).T
    sgut = np.stack([tile_gu(sgT), tile_gu(suT)]).astype(BF16)
    sdP = pack_down(np.asarray(s_down, f32).T).astype(BF16)

    gp = np.asarray(gate_proj, f32)
    up = np.asarray(up_proj, f32)
    dn = np.asarray(down_proj, f32)

    in_maps = []
    for c in range(N_CORES):
        gutc = np.stack([tile_gu(gp[c].T), tile_gu(up[c].T)]).astype(BF16)
        dpPc = pack_down(dn[c].T).astype(BF16)
        in_maps.append({
            "hidTc": hidT[c], "hidf": hidf, "gw": gw,
            "gut": gutc, "dpP": dpPc, "sgut": sgut, "sdP": sdP,
        })
    return in_maps


def _combine(results):
    f32 = np.float32
    comb = np.zeros((B, S, H), f32)
    b_ix = np.arange(B)[:, None]
    for c in range(N_CORES):
        r = results[c]
        idx = r["idxo"].astype(np.int64)
        comb[b_ix, idx] += r["w_out"].astype(f32)
    shared = np.stack([results[c]["sh_out"].astype(f32) for c in range(N_CORES)])
    return comb.transpose(0, 2, 1) + shared


def kernel(**inputs):
    ex = _get_exec()
    in_maps = _prep_in_maps(**inputs)
    results = ex.run(in_maps)
    return _combine(results).astype(np.float32)


# revision 19
# speedup vs baseline: 1.1027x; 1.1027x over previous
"""DeepseekECMoE (expert-choice MoE) Trainium2 kernel, 8-way expert-parallel.

Layout per core c (SPMD, differences only via inputs):
  - gate (f32r matmul, all batches, interleaved with shared-expert
    gate/up on PE so the 32MB hidT stream hides under compute) ->
    softmax row for expert c -> exact top-256 per batch via
    max8/max_index/match_replace (DVE, overlaps shared-expert down on
    PE) -> token gather via indirect DMA -> expert MLP (bf16 matmuls,
    erf-gelu on ACT, weights loaded once, pg/pu accumulation chains
    interleaved to pipeline the PE) -> score-weighted bf16 outputs +
    indices out.
  - shared expert for batch b=c (bf16 matmuls), bf16 output.
Host combines: scatter-add weighted expert outputs, transpose, add shared.
"""
import numpy as np
import ml_dtypes

import concourse.bass as bass
import concourse.tile as tile
from concourse import bacc, mybir
from concourse.bass2jax import install_neuronx_cc_hook, _bass_exec_p, partition_id_tensor
from concourse.masks import make_identity

B, S, H, E = 8, 1024, 1024, 8
I, ISH, CAP = 2048, 2048, 256
P = 128
HC, SC, NI, NISH = H // P, S // P, I // P, ISH // P
N_CORES = 8
dt = mybir.dt
BF16 = ml_dtypes.bfloat16

_CACHE: dict = {}


def _build_nc(act_name="Gelu"):
    nc = bacc.Bacc("TRN2", target_bir_lowering=False, debug=False,
                   num_devices=N_CORES)

    # ---- DRAM I/O ----
    hidT = nc.dram_tensor("hidT", [B, H, S], dt.float32r, kind="ExternalInput")
    hidf = nc.dram_tensor("hidf", [B * S, H], dt.bfloat16, kind="ExternalInput")
    gw = nc.dram_tensor("gw", [P, HC * E], dt.float32r, kind="ExternalInput")
    esel = nc.dram_tensor("esel", [E, 1], dt.float32r, kind="ExternalInput")
    ones8 = nc.dram_tensor("ones8", [E, 1], dt.float32r, kind="ExternalInput")
    hshb = nc.dram_tensor("hshb", [H, S], dt.bfloat16, kind="ExternalInput")
    # gate/up weights packed in i-pairs: [2(g|u), NI//2, P, 2*HC*P]
    gut = nc.dram_tensor("gut", [2, NI // 2, P, 2 * HC * P], dt.bfloat16,
                         kind="ExternalInput")
    sgut = nc.dram_tensor("sgut", [2, NISH // 2, P, 2 * HC * P], dt.bfloat16,
                          kind="ExternalInput")
    # down weights packed partition-major: [P, NI*H]
    dpP = nc.dram_tensor("dpP", [P, NI * H], dt.bfloat16, kind="ExternalInput")
    sdP = nc.dram_tensor("sdP", [P, NISH * H], dt.bfloat16, kind="ExternalInput")

    w_out = nc.dram_tensor("w_out", [B, CAP, H], dt.bfloat16, kind="ExternalOutput")
    idxo = nc.dram_tensor("idxo", [B, CAP], dt.uint32, kind="ExternalOutput")
    sh_out = nc.dram_tensor("sh_out", [S, H], dt.bfloat16, kind="ExternalOutput")

    AF = mybir.ActivationFunctionType
    ACT = getattr(AF, act_name)
    from contextlib import ExitStack
    with tile.TileContext(nc) as tc:
        with ExitStack() as ctx:
            pool = lambda name, bufs, **kw: ctx.enter_context(
                tc.tile_pool(name=name, bufs=bufs, **kw))
            pconst = pool("consts", 1)
            phts = pool("hts", 7)
            pexp = pool("exp", 2)
            pwork = pool("work", 1)
            prden = pool("rden", 1)
            proute = pool("route", 1)
            phsh = pool("hsh", 8)
            pguw = pool("guw", 5)
            pacts = pool("acts", 16)
            pdw = pool("dw", 1)
            ptokT = pool("tokT", 8)
            pgath = pool("gath", 3)
            pgel = pool("gel", 2)
            pwo = pool("wo", 4)
            psmallT = pool("smallT", 2)
            # PSUM: 8 banks total
            pps = pool("pps", 2, space="PSUM")   # gate softmax + down + transposes
            pgu = pool("pgu", 3, space="PSUM")   # pg + pu tags -> 6 banks

            # ---- constants ----
            t_gw = pconst.tile([P, HC * E], dt.float32r)
            nc.sync.dma_start(t_gw[:], gw[:])
            t_esel = pconst.tile([E, 1], dt.float32r)
            nc.sync.dma_start(t_esel[:], esel[:])
            t_ones8 = pconst.tile([E, 1], dt.float32r)
            nc.sync.dma_start(t_ones8[:], ones8[:])
            t_idb = pconst.tile([P, P], dt.bfloat16)
            make_identity(nc, t_idb[:])
            t_idf8 = pconst.tile([E, E], dt.float32)
            make_identity(nc, t_idf8[:])
            t_iotB = pconst.tile([E, 1], dt.int32)
            nc.gpsimd.iota(t_iotB[:], pattern=[[0, 1]], base=0,
                           channel_multiplier=S)
            t_iotBf = pconst.tile([E, 1], dt.float32)
            nc.vector.tensor_copy(t_iotBf[:], t_iotB[:])

            # ---- early loads: shared-expert hidden ----
            hsh = []
            for hc in range(HC):
                t = phsh.tile([P, S], dt.bfloat16, tag="hsh", name="hsh")
                nc.sync.dma_start(t[:], hshb[hc * P:(hc + 1) * P, :])
                hsh.append(t)
            sdw = pdw.tile([P, NISH * H], dt.bfloat16, tag="dw", name="sdw")

            afftile = proute.tile([E, S], dt.float32)

            def gate_batch(b):
                """Gate matmuls + softmax for batch b -> afftile[b]."""
                exp_b = pexp.tile([E, S], dt.float32r)
                pl = [pps.tile([P, 512], dt.float32, tag="pps", name="pl0"),
                      pps.tile([P, 512], dt.float32, tag="pps", name="pl1")]
                for hc in range(HC):
                    ht = phts.tile([P, S], dt.float32r)
                    nc.sync.dma_start(ht[:], hidT[b, hc * P:(hc + 1) * P, :])
                    for sblk in range(2):
                        nc.tensor.matmul(pl[sblk][:E],
                                         t_gw[:, hc * E:(hc + 1) * E],
                                         ht[:, sblk * 512:(sblk + 1) * 512],
                                         start=(hc == 0), stop=(hc == HC - 1))
                for sblk in range(2):
                    nc.scalar.activation(exp_b[:, sblk * 512:(sblk + 1) * 512],
                                         pl[sblk][:E], AF.Exp)
                rden = prden.tile([1, S], dt.float32)
                affrow = pwork.tile([1, S], dt.float32, tag="rt", name="affrow")
                for sblk in range(2):
                    sl = slice(sblk * 512, (sblk + 1) * 512)
                    pden = pps.tile([P, 512], dt.float32, tag="pps", name="pden")
                    nc.tensor.matmul(pden[:1], t_ones8[:], exp_b[:, sl],
                                     start=True, stop=True)
                    nc.vector.reciprocal(rden[:, sl], pden[:1])
                    psel = pps.tile([P, 512], dt.float32, tag="pps", name="psel")
                    nc.tensor.matmul(psel[:1], t_esel[:], exp_b[:, sl],
                                     start=True, stop=True)
                    nc.vector.tensor_mul(affrow[:, sl], psel[:1], rden[:, sl])
                nc.sync.dma_start(afftile[b:b + 1, :], affrow[:])

            def gu_block(sgu, suu, k, rhs_tiles, a):
                """One i of gate/up: interleaved pg/pu accumulation chains."""
                sg = sgu[:, k * HC * P:(k + 1) * HC * P]
                su = suu[:, k * HC * P:(k + 1) * HC * P]
                for ch in range(2):
                    col = ch * 512
                    pg = pgu.tile([P, 512], dt.float32, tag="pg", name="pg")
                    pu = pgu.tile([P, 512], dt.float32, tag="pu", name="pu")
                    for hc in range(HC):
                        rhs = rhs_tiles[hc][:, col:col + 512]
                        nc.tensor.matmul(pg[:], sg[:, hc * P:(hc + 1) * P], rhs,
                                         start=(hc == 0), stop=(hc == HC - 1))
                        nc.tensor.matmul(pu[:], su[:, hc * P:(hc + 1) * P], rhs,
                                         start=(hc == 0), stop=(hc == HC - 1))
                    gel = pgel.tile([P, 512], dt.float32)
                    nc.scalar.activation(gel[:], pg[:], ACT)
                    nc.vector.tensor_mul(a[:, col:col + 512], gel[:], pu[:])

            # ---- interleaved: shared gate/up blocks + gate batches ----
            actsh = []
            for j in range(NISH // 2):
                sgu = pguw.tile([P, 2 * HC * P], dt.bfloat16, name="sgu")
                nc.sync.dma_start(sgu[:], sgut[0, j])
                suu = pguw.tile([P, 2 * HC * P], dt.bfloat16, name="suu")
                nc.sync.dma_start(suu[:], sgut[1, j])
                # trickle in the shared down weights (needed after g/u)
                nc.sync.dma_start(sdw[:, j * 2 * H:(j + 1) * 2 * H],
                                  sdP[:, j * 2 * H:(j + 1) * 2 * H])
                for k in range(2):
                    a = pacts.tile([P, S], dt.bfloat16, tag="acts", name="acts")
                    gu_block(sgu, suu, k, hsh, a)
                    actsh.append(a)
                gate_batch(j)

            # ---- shared expert down (PE; overlaps top-k on DVE) ----
            for sblk in range(SC):
                for hh in range(2):
                    pd = pps.tile([P, 512], dt.float32, tag="pps", name="pd")
                    for ic in range(NISH):
                        nc.tensor.matmul(
                            pd[:], actsh[ic][:, sblk * P:(sblk + 1) * P],
                            sdw[:, ic * H + hh * 512:ic * H + hh * 512 + 512],
                            start=(ic == 0), stop=(ic == NISH - 1))
                    sho = pwo.tile([P, 512], dt.bfloat16, tag="wo", name="wo")
                    nc.scalar.copy(sho[:], pd[:])
                    nc.sync.dma_start(
                        sh_out[sblk * P:(sblk + 1) * P, hh * 512:(hh + 1) * 512],
                        sho[:])

            # ---- top-k (serial DVE chain) ----
            t_scores = proute.tile([E, CAP], dt.float32)
            t_idxu = proute.tile([E, CAP], dt.uint32)
            t_idxf = proute.tile([E, CAP], dt.float32)
            t_idxg = proute.tile([E, CAP], dt.float32)
            for i in range(CAP // 8):
                sc8 = t_scores[:, i * 8:(i + 1) * 8]
                nc.vector.max(sc8, afftile[:])
                nc.vector.max_index(t_idxu[:, i * 8:(i + 1) * 8], sc8, afftile[:])
                nc.vector.match_replace(afftile[:], sc8, afftile[:], -1e30)
            nc.sync.dma_start(idxo[:], t_idxu[:])
            nc.vector.tensor_copy(t_idxf[:], t_idxu[:])
            nc.vector.tensor_scalar(t_idxg[:], t_idxf[:], t_iotBf[:, :1],
                                    None, mybir.AluOpType.add)

            # transpose scores + global indices to per-token columns
            scT = [psmallT.tile([P, E], dt.float32, tag="scT", name="scT")
                   for _ in range(2)]
            idxT = [psmallT.tile([P, E], dt.uint32, tag="idxT", name="idxT")
                    for _ in range(2)]
            for half in range(2):
                ptp = pps.tile([P, 512], dt.float32, tag="pps", name="ptp")
                nc.tensor.transpose(ptp[:, :E],
                                    t_scores[:, half * P:(half + 1) * P],
                                    t_idf8[:])
                nc.vector.tensor_copy(scT[half][:], ptp[:, :E])
                ptq = pps.tile([P, 512], dt.float32, tag="pps", name="ptq")
                nc.tensor.transpose(ptq[:, :E],
                                    t_idxg[:, half * P:(half + 1) * P],
                                    t_idf8[:])
                nc.vector.tensor_copy(idxT[half][:], ptq[:, :E])

            # ---- routed: two half-passes of 1024 tokens each ----
            dpw = pdw.tile([P, NI * H], dt.bfloat16, tag="dw", name="dpw")
            for k in range(8):
                nc.sync.dma_start(dpw[:, k * 2 * H:(k + 1) * 2 * H],
                                  dpP[:, k * 2 * H:(k + 1) * 2 * H])
            for half in range(2):
                # dispatch: indirect gather + PE transpose into tokT
                tokT = [ptokT.tile([P, 8 * P], dt.bfloat16, tag="tokT",
                                   name="tokT") for _ in range(HC)]
                for b in range(B):
                    g = pgath.tile([P, H], dt.bfloat16, tag="g", name="g")
                    nc.gpsimd.indirect_dma_start(
                        out=g[:], out_offset=None, in_=hidf[:],
                        in_offset=bass.IndirectOffsetOnAxis(
                            ap=idxT[half][:, b:b + 1], axis=0))
                    for hc in range(HC):
                        ptp = pps.tile([P, 512], dt.bfloat16, tag="pps",
                                       name="ptr")
                        nc.tensor.transpose(ptp[:, :P],
                                            g[:, hc * P:(hc + 1) * P], t_idb[:])
                        nc.scalar.copy(tokT[hc][:, b * P:(b + 1) * P],
                                       ptp[:, :P])

                # gate/up (weights streamed; second half streams again)
                actT = []
                for j in range(NI // 2):
                    sgu = pguw.tile([P, 2 * HC * P], dt.bfloat16, name="sgu")
                    nc.sync.dma_start(sgu[:], gut[0, j])
                    suu = pguw.tile([P, 2 * HC * P], dt.bfloat16, name="suu")
                    nc.sync.dma_start(suu[:], gut[1, j])
                    for k in range(2):
                        a = pacts.tile([P, S], dt.bfloat16, tag="acts",
                                       name="acts")
                        gu_block(sgu, suu, k, tokT, a)
                        actT.append(a)

                # down + score weighting
                for b in range(B):
                    for hh in range(2):
                        pd = pps.tile([P, 512], dt.float32, tag="pps", name="pd")
                        for ic in range(NI):
                            nc.tensor.matmul(
                                pd[:], actT[ic][:, b * P:(b + 1) * P],
                                dpw[:, ic * H + hh * 512:ic * H + hh * 512 + 512],
                                start=(ic == 0), stop=(ic == NI - 1))
                        wo = pwo.tile([P, 512], dt.bfloat16, tag="wo", name="wo")
                        nc.vector.tensor_scalar(wo[:], pd[:],
                                                scT[half][:, b:b + 1],
                                                None, mybir.AluOpType.mult)
                        nc.sync.dma_start(
                            w_out[b, half * P:(half + 1) * P,
                                  hh * 512:(hh + 1) * 512], wo[:])

    nc.compile()
    return nc


class _Exec:
    """Cached multi-core PJRT executor (mirrors bass2jax.run_bass_via_pjrt)."""

    def __init__(self, nc):
        import jax
        from jax.sharding import Mesh, PartitionSpec
        from jax.experimental.shard_map import shard_map

        install_neuronx_cc_hook()
        self.nc = nc
        in_names, out_names, out_avals = [], [], []
        partition_name = (nc.partition_id_tensor.name
                          if nc.partition_id_tensor else None)
        for alloc in nc.m.functions[0].allocations:
            if not isinstance(alloc, mybir.MemoryLocationSet):
                continue
            if alloc.kind not in ("ExternalInput", "ExternalOutput"):
                continue
            name = alloc.memorylocations[0].name
            if alloc.kind == "ExternalInput":
                if name != partition_name:
                    in_names.append(name)
            elif alloc.kind == "ExternalOutput":
                out_names.append(name)
                out_avals.append(jax.core.ShapedArray(
                    tuple(alloc.tensor_shape), mybir.dt.np(alloc.dtype)))
        self.in_names, self.out_names, self.out_avals = in_names, out_names, out_avals
        self.partition_name = partition_name
        n_params = len(in_names)
        n_outs = len(out_names)
        all_in_names = list(in_names) + list(out_names)
        if partition_name is not None:
            all_in_names.append(partition_name)

        def _body(*args):
            operands = list(args)
            if partition_name is not None:
                operands.append(partition_id_tensor())
            outs = _bass_exec_p.bind(
                *operands,
                out_avals=tuple(out_avals),
                in_names=tuple(all_in_names),
                out_names=tuple(out_names),
                lowering_input_output_aliases=(),
                sim_require_finite=True,
                sim_require_nnan=True,
                nc=nc,
            )
            return tuple(outs)

        devices = jax.devices()[:N_CORES]
        mesh = Mesh(np.asarray(devices), ("core",))
        in_specs = (PartitionSpec("core"),) * (n_params + n_outs)
        out_specs = (PartitionSpec("core"),) * n_outs
        self.sharded = jax.jit(
            shard_map(_body, mesh=mesh, in_specs=in_specs, out_specs=out_specs,
                      check_rep=False),
            donate_argnums=tuple(range(n_params, n_params + n_outs)),
            keep_unused=True,
        )

    def concat_inputs(self, in_maps):
        return [
            np.concatenate([np.asarray(in_maps[c][name]) for c in range(N_CORES)],
                           axis=0)
            for name in self.in_names
        ]

    def zero_outs(self):
        return [np.zeros((N_CORES * a.shape[0], *a.shape[1:]), a.dtype)
                for a in self.out_avals]

    def run_raw(self, concat_in):
        return self.sharded(*concat_in, *self.zero_outs())

    def run(self, in_maps):
        out_arrs = self.run_raw(self.concat_inputs(in_maps))
        return [
            {name: np.asarray(out_arrs[i]).reshape(N_CORES, *self.out_avals[i].shape)[c]
             for i, name in enumerate(self.out_names)}
            for c in range(N_CORES)
        ]


def _get_exec():
    if "exec" not in _CACHE:
        _CACHE["exec"] = _Exec(_build_nc())
    return _CACHE["exec"]


def _prep_in_maps(hidden_states, gate_w, gate_proj, up_proj, down_proj,
                  s_gate, s_up, s_down):
    f32 = np.float32
    hid = np.ascontiguousarray(hidden_states, dtype=f32)
    hidT = np.ascontiguousarray(hid.transpose(0, 2, 1))
    hidf = np.ascontiguousarray(hid.reshape(B * S, H)).astype(BF16)
    gw = np.ascontiguousarray(
        np.asarray(gate_w, f32).reshape(HC, P, E).transpose(1, 0, 2).reshape(P, HC * E))
    ones8 = np.ones((E, 1), f32)

    def tile_gu(gT):  # gT [H, X] -> [X//(2P), P, 2*HC*P]  (i-pairs packed)
        X = gT.shape[1]
        t = gT.reshape(HC, P, X // P, P).transpose(2, 1, 0, 3)  # [X/P, P, HC, P]
        t = t.reshape(X // P, P, HC * P)
        return np.ascontiguousarray(
            t.reshape(X // (2 * P), 2, P, HC * P).transpose(0, 2, 1, 3)
            .reshape(X // (2 * P), P, 2 * HC * P))

    def pack_down(dT):  # dT [X, H] -> [P, (X//P)*H] partition-major
        X = dT.shape[0]
        return np.ascontiguousarray(
            dT.reshape(X // P, P, H).transpose(1, 0, 2).reshape(P, (X // P) * H))

    sgT = np.asarray(s_gate, f32).T  # [H, ISH]
    suT = np.asarray(s_up, f32).T
    sgut = np.stack([tile_gu(sgT), tile_gu(suT)]).astype(BF16)
    sdP = pack_down(np.asarray(s_down, f32).T).astype(BF16)

    gp = np.asarray(gate_proj, f32)
    up = np.asarray(up_proj, f32)
    dn = np.asarray(down_proj, f32)

    in_maps = []
    for c in range(N_CORES):
        gutc = np.stack([tile_gu(gp[c].T), tile_gu(up[c].T)]).astype(BF16)
        dpPc = pack_down(dn[c].T).astype(BF16)
        es = np.zeros((E, 1), f32)
        es[c, 0] = 1.0
        in_maps.append({
            "hidT": hidT, "hidf": hidf, "gw": gw, "esel": es, "ones8": ones8,
            "hshb": hidT[c].astype(BF16),
            "gut": gutc, "dpP": dpPc, "sgut": sgut, "sdP": sdP,
        })
    return in_maps


def _combine(results):
    f32 = np.float32
    comb = np.zeros((B, S, H), f32)
    b_ix = np.arange(B)[:, None]
    for c in range(N_CORES):
        r = results[c]
        idx = r["idxo"].astype(np.int64)
        comb[b_ix, idx] += r["w_out"].astype(f32)
    shared = np.stack([results[c]["sh_out"].astype(f32) for c in range(N_CORES)])
    return comb.transpose(0, 2, 1) + shared


def kernel(**inputs):
    ex = _get_exec()
    in_maps = _prep_in_maps(**inputs)
    results = ex.run(in_maps)
    return _combine(results).astype(np.float32)


# revision 20
# speedup vs baseline: 1.1514x; 1.0442x over previous
"""DeepseekECMoE (expert-choice MoE) Trainium2 kernel, 8-way expert-parallel.

Layout per core c (SPMD, differences only via inputs):
  - gate (f32r matmul, all batches, interleaved with shared-expert
    gate/up on PE so the 32MB hidT stream hides under compute) ->
    softmax row for expert c -> exact top-256 per batch via
    max8/max_index/match_replace (DVE, overlaps shared-expert down on
    PE) -> token gather via indirect DMA -> expert MLP (bf16 matmuls,
    erf-gelu on ACT, weights loaded once, pg/pu accumulation chains
    interleaved to pipeline the PE) -> score-weighted bf16 outputs +
    indices out.
  - shared expert for batch b=c (bf16 matmuls), bf16 output.
Host combines: scatter-add weighted expert outputs, transpose, add shared.
"""
import numpy as np
import ml_dtypes

import concourse.bass as bass
import concourse.tile as tile
from concourse import bacc, mybir
from concourse.bass2jax import install_neuronx_cc_hook, _bass_exec_p, partition_id_tensor
from concourse.masks import make_identity

B, S, H, E = 8, 1024, 1024, 8
I, ISH, CAP = 2048, 2048, 256
P = 128
HC, SC, NI, NISH = H // P, S // P, I // P, ISH // P
N_CORES = 8
dt = mybir.dt
BF16 = ml_dtypes.bfloat16

_CACHE: dict = {}


def _build_nc(act_name="Gelu"):
    nc = bacc.Bacc("TRN2", target_bir_lowering=False, debug=False,
                   num_devices=N_CORES)

    # ---- DRAM I/O ----
    hidT = nc.dram_tensor("hidT", [B, H, S], dt.float32r, kind="ExternalInput")
    hidf = nc.dram_tensor("hidf", [B * S, H], dt.bfloat16, kind="ExternalInput")
    gw = nc.dram_tensor("gw", [P, HC * E], dt.float32r, kind="ExternalInput")
    esel = nc.dram_tensor("esel", [E, 1], dt.float32r, kind="ExternalInput")
    ones8 = nc.dram_tensor("ones8", [E, 1], dt.float32r, kind="ExternalInput")
    hshb = nc.dram_tensor("hshb", [H, S], dt.bfloat16, kind="ExternalInput")
    # gate/up weights packed in i-pairs: [2(g|u), NI//2, P, 2*HC*P]
    gut = nc.dram_tensor("gut", [2, NI // 2, P, 2 * HC * P], dt.bfloat16,
                         kind="ExternalInput")
    sgut = nc.dram_tensor("sgut", [2, NISH // 2, P, 2 * HC * P], dt.bfloat16,
                          kind="ExternalInput")
    # down weights packed partition-major: [P, NI*H]
    dpP = nc.dram_tensor("dpP", [P, NI * H], dt.bfloat16, kind="ExternalInput")
    sdP = nc.dram_tensor("sdP", [P, NISH * H], dt.bfloat16, kind="ExternalInput")

    w_out = nc.dram_tensor("w_out", [B, CAP, H], dt.bfloat16, kind="ExternalOutput")
    idxo = nc.dram_tensor("idxo", [B, CAP], dt.uint32, kind="ExternalOutput")
    sh_out = nc.dram_tensor("sh_out", [S, H], dt.bfloat16, kind="ExternalOutput")

    AF = mybir.ActivationFunctionType
    ACT = getattr(AF, act_name)
    from contextlib import ExitStack
    with tile.TileContext(nc) as tc:
        with ExitStack() as ctx:
            pool = lambda name, bufs, **kw: ctx.enter_context(
                tc.tile_pool(name=name, bufs=bufs, **kw))
            pconst = pool("consts", 1)
            phts = pool("hts", 7)
            pexp = pool("exp", 2)
            pwork = pool("work", 1)
            prden = pool("rden", 1)
            proute = pool("route", 1)
            phsh = pool("hsh", 8)
            pguw = pool("guw", 5)
            pacts = pool("acts", 16)
            pdw = pool("dw", 1)
            ptokT = pool("tokT", 8)
            pgath = pool("gath", 3)
            pgel = pool("gel", 2)
            pwo = pool("wo", 4)
            psmallT = pool("smallT", 2)
            # PSUM: 8 banks total
            pps = pool("pps", 2, space="PSUM")   # gate softmax + down + transposes
            pgu = pool("pgu", 3, space="PSUM")   # pg + pu tags -> 6 banks

            # ---- constants ----
            t_gw = pconst.tile([P, HC * E], dt.float32r)
            nc.sync.dma_start(t_gw[:], gw[:])
            t_esel = pconst.tile([E, 1], dt.float32r)
            nc.sync.dma_start(t_esel[:], esel[:])
            t_ones8 = pconst.tile([E, 1], dt.float32r)
            nc.sync.dma_start(t_ones8[:], ones8[:])
            t_idb = pconst.tile([P, P], dt.bfloat16)
            make_identity(nc, t_idb[:])
            t_idf8 = pconst.tile([E, E], dt.float32)
            make_identity(nc, t_idf8[:])
            t_iotB = pconst.tile([E, 1], dt.int32)
            nc.gpsimd.iota(t_iotB[:], pattern=[[0, 1]], base=0,
                           channel_multiplier=S)
            t_iotBf = pconst.tile([E, 1], dt.float32)
            nc.vector.tensor_copy(t_iotBf[:], t_iotB[:])

            # ---- early loads: shared-expert hidden ----
            hsh = []
            for hc in range(HC):
                t = phsh.tile([P, S], dt.bfloat16, tag="hsh", name="hsh")
                nc.sync.dma_start(t[:], hshb[hc * P:(hc + 1) * P, :])
                hsh.append(t)
            sdw = pdw.tile([P, NISH * H], dt.bfloat16, tag="dw", name="sdw")

            afftile = proute.tile([E, S], dt.float32)

            def gate_batch(b):
                """Gate matmuls + softmax for batch b -> afftile[b]."""
                exp_b = pexp.tile([E, S], dt.float32r)
                pl = [pps.tile([P, 512], dt.float32, tag="pps", name="pl0"),
                      pps.tile([P, 512], dt.float32, tag="pps", name="pl1")]
                for hc in range(HC):
                    ht = phts.tile([P, S], dt.float32r)
                    nc.sync.dma_start(ht[:], hidT[b, hc * P:(hc + 1) * P, :])
                    for sblk in range(2):
                        nc.tensor.matmul(pl[sblk][:E],
                                         t_gw[:, hc * E:(hc + 1) * E],
                                         ht[:, sblk * 512:(sblk + 1) * 512],
                                         start=(hc == 0), stop=(hc == HC - 1))
                for sblk in range(2):
                    nc.scalar.activation(exp_b[:, sblk * 512:(sblk + 1) * 512],
                                         pl[sblk][:E], AF.Exp)
                rden = prden.tile([1, S], dt.float32)
                affrow = pwork.tile([1, S], dt.float32, tag="rt", name="affrow")
                for sblk in range(2):
                    sl = slice(sblk * 512, (sblk + 1) * 512)
                    pden = pps.tile([P, 512], dt.float32, tag="pps", name="pden")
                    nc.tensor.matmul(pden[:1], t_ones8[:], exp_b[:, sl],
                                     start=True, stop=True)
                    nc.vector.reciprocal(rden[:, sl], pden[:1])
                    psel = pps.tile([P, 512], dt.float32, tag="pps", name="psel")
                    nc.tensor.matmul(psel[:1], t_esel[:], exp_b[:, sl],
                                     start=True, stop=True)
                    nc.vector.tensor_mul(affrow[:, sl], psel[:1], rden[:, sl])
                nc.sync.dma_start(afftile[b:b + 1, :], affrow[:])

            def gu_block(sgu, suu, k, rhs_tiles, a):
                """One i of gate/up: interleaved pg/pu accumulation chains."""
                sg = sgu[:, k * HC * P:(k + 1) * HC * P]
                su = suu[:, k * HC * P:(k + 1) * HC * P]
                for ch in range(2):
                    col = ch * 512
                    pg = pgu.tile([P, 512], dt.float32, tag="pg", name="pg")
                    pu = pgu.tile([P, 512], dt.float32, tag="pu", name="pu")
                    for hc in range(HC):
                        rhs = rhs_tiles[hc][:, col:col + 512]
                        nc.tensor.matmul(pg[:], sg[:, hc * P:(hc + 1) * P], rhs,
                                         start=(hc == 0), stop=(hc == HC - 1))
                        nc.tensor.matmul(pu[:], su[:, hc * P:(hc + 1) * P], rhs,
                                         start=(hc == 0), stop=(hc == HC - 1))
                    gel = pgel.tile([P, 512], dt.float32)
                    nc.scalar.activation(gel[:], pg[:], ACT)
                    nc.vector.tensor_mul(a[:, col:col + 512], gel[:], pu[:])

            # ---- interleaved: shared gate/up blocks + gate batches ----
            actsh = []
            for j in range(NISH // 2):
                sgu = pguw.tile([P, 2 * HC * P], dt.bfloat16, name="sgu")
                nc.sync.dma_start(sgu[:], sgut[0, j])
                suu = pguw.tile([P, 2 * HC * P], dt.bfloat16, name="suu")
                nc.sync.dma_start(suu[:], sgut[1, j])
                # trickle in the shared down weights (needed after g/u)
                nc.sync.dma_start(sdw[:, j * 2 * H:(j + 1) * 2 * H],
                                  sdP[:, j * 2 * H:(j + 1) * 2 * H])
                for k in range(2):
                    a = pacts.tile([P, S], dt.bfloat16, tag="acts", name="acts")
                    gu_block(sgu, suu, k, hsh, a)
                    actsh.append(a)
                if j < 4:
                    gate_batch(2 * j)
                    gate_batch(2 * j + 1)

            # ---- shared expert down (PE; overlaps top-k on DVE) ----
            for sblk in range(SC):
                for hh in range(2):
                    pd = pps.tile([P, 512], dt.float32, tag="pps", name="pd")
                    for ic in range(NISH):
                        nc.tensor.matmul(
                            pd[:], actsh[ic][:, sblk * P:(sblk + 1) * P],
                            sdw[:, ic * H + hh * 512:ic * H + hh * 512 + 512],
                            start=(ic == 0), stop=(ic == NISH - 1))
                    sho = pwo.tile([P, 512], dt.bfloat16, tag="wo", name="wo")
                    nc.scalar.copy(sho[:], pd[:])
                    nc.sync.dma_start(
                        sh_out[sblk * P:(sblk + 1) * P, hh * 512:(hh + 1) * 512],
                        sho[:])

            # ---- top-k: two 16-iteration groups, one per 128-token half ----
            t_scores = proute.tile([E, CAP], dt.float32)
            t_idxu = proute.tile([E, CAP], dt.uint32)
            t_idxf = proute.tile([E, CAP], dt.float32)
            t_idxg = proute.tile([E, CAP], dt.float32)
            scT = [psmallT.tile([P, E], dt.float32, tag="scT", name="scT")
                   for _ in range(2)]
            idxT = [psmallT.tile([P, E], dt.uint32, tag="idxT", name="idxT")
                    for _ in range(2)]

            def topk_iter(i):
                sc8 = t_scores[:, i * 8:(i + 1) * 8]
                nc.vector.max(sc8, afftile[:])
                nc.vector.max_index(t_idxu[:, i * 8:(i + 1) * 8], sc8, afftile[:])
                nc.vector.match_replace(afftile[:], sc8, afftile[:], -1e30)

            def finish_half(half):
                sl = slice(half * P, (half + 1) * P)
                nc.sync.dma_start(idxo[:, sl], t_idxu[:, sl])
                nc.vector.tensor_copy(t_idxf[:, sl], t_idxu[:, sl])
                nc.vector.tensor_scalar(t_idxg[:, sl], t_idxf[:, sl],
                                        t_iotBf[:, :1],
                                        None, mybir.AluOpType.add)
                ptp = pps.tile([P, 512], dt.float32, tag="pps", name="ptp")
                nc.tensor.transpose(ptp[:, :E], t_scores[:, sl], t_idf8[:])
                nc.vector.tensor_copy(scT[half][:], ptp[:, :E])
                ptq = pps.tile([P, 512], dt.float32, tag="pps", name="ptq")
                nc.tensor.transpose(ptq[:, :E], t_idxg[:, sl], t_idf8[:])
                nc.vector.tensor_copy(idxT[half][:], ptq[:, :E])

            for i in range(16):
                topk_iter(i)
            finish_half(0)

            # ---- routed: two half-passes of 1024 tokens each ----
            dpw = pdw.tile([P, NI * H], dt.bfloat16, tag="dw", name="dpw")
            for k in range(8):
                nc.sync.dma_start(dpw[:, k * 2 * H:(k + 1) * 2 * H],
                                  dpP[:, k * 2 * H:(k + 1) * 2 * H])
            for half in range(2):
                # dispatch: indirect gather + PE transpose into tokT
                tokT = [ptokT.tile([P, 8 * P], dt.bfloat16, tag="tokT",
                                   name="tokT") for _ in range(HC)]
                for b in range(B):
                    g = pgath.tile([P, H], dt.bfloat16, tag="g", name="g")
                    nc.gpsimd.indirect_dma_start(
                        out=g[:], out_offset=None, in_=hidf[:],
                        in_offset=bass.IndirectOffsetOnAxis(
                            ap=idxT[half][:, b:b + 1], axis=0))
                    for hc in range(HC):
                        ptp = pps.tile([P, 512], dt.bfloat16, tag="pps",
                                       name="ptr")
                        nc.tensor.transpose(ptp[:, :P],
                                            g[:, hc * P:(hc + 1) * P], t_idb[:])
                        nc.scalar.copy(tokT[hc][:, b * P:(b + 1) * P],
                                       ptp[:, :P])

                # gate/up (weights streamed; second half streams again)
                actT = []
                for j in range(NI // 2):
                    sgu = pguw.tile([P, 2 * HC * P], dt.bfloat16, name="sgu")
                    nc.sync.dma_start(sgu[:], gut[0, j])
                    suu = pguw.tile([P, 2 * HC * P], dt.bfloat16, name="suu")
                    nc.sync.dma_start(suu[:], gut[1, j])
                    for k in range(2):
                        a = pacts.tile([P, S], dt.bfloat16, tag="acts",
                                       name="acts")
                        gu_block(sgu, suu, k, tokT, a)
                        actT.append(a)

                # down + score weighting; half 0 hides top-k iters 16-31
                for b in range(B):
                    for hh in range(2):
                        pd = pps.tile([P, 512], dt.float32, tag="pps", name="pd")
                        for ic in range(NI):
                            nc.tensor.matmul(
                                pd[:], actT[ic][:, b * P:(b + 1) * P],
                                dpw[:, ic * H + hh * 512:ic * H + hh * 512 + 512],
                                start=(ic == 0), stop=(ic == NI - 1))
                        wo = pwo.tile([P, 512], dt.bfloat16, tag="wo", name="wo")
                        nc.vector.tensor_scalar(wo[:], pd[:],
                                                scT[half][:, b:b + 1],
                                                None, mybir.AluOpType.mult)
                        nc.sync.dma_start(
                            w_out[b, half * P:(half + 1) * P,
                                  hh * 512:(hh + 1) * 512], wo[:])
                        if half == 0:
                            topk_iter(16 + 2 * b + hh)
                if half == 0:
                    finish_half(1)

    nc.compile()
    return nc


class _Exec:
    """Cached multi-core PJRT executor (mirrors bass2jax.run_bass_via_pjrt)."""

    def __init__(self, nc):
        import jax
        from jax.sharding import Mesh, PartitionSpec
        from jax.experimental.shard_map import shard_map

        install_neuronx_cc_hook()
        self.nc = nc
        in_names, out_names, out_avals = [], [], []
        partition_name = (nc.partition_id_tensor.name
                          if nc.partition_id_tensor else None)
        for alloc in nc.m.functions[0].allocations:
            if not isinstance(alloc, mybir.MemoryLocationSet):
                continue
            if alloc.kind not in ("ExternalInput", "ExternalOutput"):
                continue
            name = alloc.memorylocations[0].name
            if alloc.kind == "ExternalInput":
                if name != partition_name:
                    in_names.append(name)
            elif alloc.kind == "ExternalOutput":
                out_names.append(name)
                out_avals.append(jax.core.ShapedArray(
                    tuple(alloc.tensor_shape), mybir.dt.np(alloc.dtype)))
        self.in_names, self.out_names, self.out_avals = in_names, out_names, out_avals
        self.partition_name = partition_name
        n_params = len(in_names)
        n_outs = len(out_names)
        all_in_names = list(in_names) + list(out_names)
        if partition_name is not None:
            all_in_names.append(partition_name)

        def _body(*args):
            operands = list(args)
            if partition_name is not None:
                operands.append(partition_id_tensor())
            outs = _bass_exec_p.bind(
                *operands,
                out_avals=tuple(out_avals),
                in_names=tuple(all_in_names),
                out_names=tuple(out_names),
                lowering_input_output_aliases=(),
                sim_require_finite=True,
                sim_require_nnan=True,
                nc=nc,
            )
            return tuple(outs)

        devices = jax.devices()[:N_CORES]
        mesh = Mesh(np.asarray(devices), ("core",))
        in_specs = (PartitionSpec("core"),) * (n_params + n_outs)
        out_specs = (PartitionSpec("core"),) * n_outs
        self.sharded = jax.jit(
            shard_map(_body, mesh=mesh, in_specs=in_specs, out_specs=out_specs,
                      check_rep=False),
            donate_argnums=tuple(range(n_params, n_params + n_outs)),
            keep_unused=True,
        )

    def concat_inputs(self, in_maps):
        return [
            np.concatenate([np.asarray(in_maps[c][name]) for c in range(N_CORES)],
                           axis=0)
            for name in self.in_names
        ]

    def zero_outs(self):
        return [np.zeros((N_CORES * a.shape[0], *a.shape[1:]), a.dtype)
                for a in self.out_avals]

    def run_raw(self, concat_in):
        return self.sharded(*concat_in, *self.zero_outs())

    def run(self, in_maps):
        out_arrs = self.run_raw(self.concat_inputs(in_maps))
        return [
            {name: np.asarray(out_arrs[i]).reshape(N_CORES, *self.out_avals[i].shape)[c]
             for i, name in enumerate(self.out_names)}
            for c in range(N_CORES)
        ]


def _get_exec():
    if "exec" not in _CACHE:
        _CACHE["exec"] = _Exec(_build_nc())
    return _CACHE["exec"]


def _prep_in_maps(hidden_states, gate_w, gate_proj, up_proj, down_proj,
                  s_gate, s_up, s_down):
    f32 = np.float32
    hid = np.ascontiguousarray(hidden_states, dtype=f32)
    hidT = np.ascontiguousarray(hid.transpose(0, 2, 1))
    hidf = np.ascontiguousarray(hid.reshape(B * S, H)).astype(BF16)
    gw = np.ascontiguousarray(
        np.asarray(gate_w, f32).reshape(HC, P, E).transpose(1, 0, 2).reshape(P, HC * E))
    ones8 = np.ones((E, 1), f32)

    def tile_gu(gT):  # gT [H, X] -> [X//(2P), P, 2*HC*P]  (i-pairs packed)
        X = gT.shape[1]
        t = gT.reshape(HC, P, X // P, P).transpose(2, 1, 0, 3)  # [X/P, P, HC, P]
        t = t.reshape(X // P, P, HC * P)
        return np.ascontiguousarray(
            t.reshape(X // (2 * P), 2, P, HC * P).transpose(0, 2, 1, 3)
            .reshape(X // (2 * P), P, 2 * HC * P))

    def pack_down(dT):  # dT [X, H] -> [P, (X//P)*H] partition-major
        X = dT.shape[0]
        return np.ascontiguousarray(
            dT.reshape(X // P, P, H).transpose(1, 0, 2).reshape(P, (X // P) * H))

    sgT = np.asarray(s_gate, f32).T  # [H, ISH]
    suT = np.asarray(s_up, f32).T
    sgut = np.stack([tile_gu(sgT), tile_gu(suT)]).astype(BF16)
    sdP = pack_down(np.asarray(s_down, f32).T).astype(BF16)

    gp = np.asarray(gate_proj, f32)
    up = np.asarray(up_proj, f32)
    dn = np.asarray(down_proj, f32)

    in_maps = []
    for c in range(N_CORES):
        gutc = np.stack([tile_gu(gp[c].T), tile_gu(up[c].T)]).astype(BF16)
        dpPc = pack_down(dn[c].T).astype(BF16)
        es = np.zeros((E, 1), f32)
        es[c, 0] = 1.0
        in_maps.append({
            "hidT": hidT, "hidf": hidf, "gw": gw, "esel": es, "ones8": ones8,
            "hshb": hidT[c].astype(BF16),
            "gut": gutc, "dpP": dpPc, "sgut": sgut, "sdP": sdP,
        })
    return in_maps


def _combine(results):
    f32 = np.float32
    comb = np.zeros((B, S, H), f32)
    b_ix = np.arange(B)[:, None]
    for c in range(N_CORES):
        r = results[c]
        idx = r["idxo"].astype(np.int64)
        comb[b_ix, idx] += r["w_out"].astype(f32)
    shared = np.stack([results[c]["sh_out"].astype(f32) for c in range(N_CORES)])
    return comb.transpose(0, 2, 1) + shared


def kernel(**inputs):
    ex = _get_exec()
    in_maps = _prep_in_maps(**inputs)
    results = ex.run(in_maps)
    return _combine(results).astype(np.float32)
